# revision 2
# baseline (speedup 1.0000x reference)
"""Trainium2 Bass kernel for nn_Bottleneck_DCNv3 (8-core SPMD).

Strategy: data-parallel over pixels (2 samples x 4 row-blocks of 16 rows, one
block per NeuronCore; per-core inputs are host-sliced shards, outputs are
host-concatenated -- no collectives).

The DCNv3 deformable bilinear gather + mask blend is reformulated as a banded
matrix multiply per 128-pixel out-tile: blend = M @ window(xpw), where
xpw = cv1_out @ (in_w @ out_w) is the output-projected image and M's 81
nonzeros/row (9x9 bins) are per-pixel "tent" products
K[u,v] = sum_p softmax-mask_p * tent(offy_p - (u-dy_p)) * tent(offx_p - (v-dx_p)),
computed with replicated-column GEMMs + ACT ops, scattered into M in DRAM via
strided (diagonal) DMA descriptors, then consumed by TensorE.

Validity/borders are handled by a zero ring of width 4 around each shard and
a per-core interior mask that also carries the input_proj bias (extra GEMM
K-row), reproducing grid_sample zero-padding semantics exactly.
"""
import os
import sys
from contextlib import ExitStack

import numpy as np

if '/opt/trn_rl_repo' not in sys.path:
    sys.path.insert(0, '/opt/trn_rl_repo')

import concourse.bass as bass
import concourse.bacc as bacc
import concourse.tile as tile
from concourse import mybir
from concourse._compat import with_exitstack
from concourse.masks import make_identity
from concourse.bass_utils import run_bass_kernel_spmd

AF = mybir.ActivationFunctionType
OP = mybir.AluOpType
FP = mybir.dt.float32

C = 256
H = W = 64
N = 2
ROWS = 16
YR = 24
XR = 26
WPAD = 72
YF = YR * WPAD          # 1728
XF = XR * WPAD          # 1872
XBUF = 1 + XF + 7       # 1880
PX = ROWS * 64          # 1024
NT = 7
NB = 9
NTILE = PX // 128       # 8
QW = 10 * WPAD          # 720 window px per out-tile
LN_EPS = 1e-5

LAST_EXEC_NS = None
LAST_RESULTS = None


# ---------------------------------------------------------------- host prep
def host_consts(inputs):
    """Shared (core-independent) constant tensors."""
    f32 = lambda a: np.ascontiguousarray(a, np.float32)
    cons = {}
    w1 = np.asarray(inputs['w1'], np.float32)  # (co, ci, 3, 3)
    w1t = np.zeros((128, 9 * 2 * 256), np.float32)
    for tap in range(9):
        for cic in range(2):
            blk = w1[:, cic * 128:(cic + 1) * 128, tap // 3, tap % 3].T
            w1t[:, (tap * 2 + cic) * 256:(tap * 2 + cic + 1) * 256] = blk
    cons['w1t'] = w1t

    s1 = inputs['bn1_g'] / np.sqrt(inputs['bn1_v'] + 1e-5)
    cons['bn1s'] = f32(np.stack([s1[:128], s1[128:]], 1))
    b1 = inputs['bn1_b'] - inputs['bn1_m'] * s1
    cons['bn1b'] = f32(np.stack([b1[:128], b1[128:]], 1))

    W2 = np.asarray(inputs['in_w'], np.float32) @ np.asarray(inputs['out_w'], np.float32)
    w2c = np.zeros((128, 2 * 256), np.float32)
    for cic in range(2):
        w2c[:, cic * 256:(cic + 1) * 256] = W2[cic * 128:(cic + 1) * 128, :]
    cons['w2c'] = w2c
    cons['inbw2'] = f32((np.asarray(inputs['in_b']) @ np.asarray(inputs['out_w']))[None, :])

    dw = np.asarray(inputs['dw_w'], np.float32).reshape(C, 9)
    cons['dww'] = f32(np.concatenate([dw[:128], dw[128:]], 1))
    cons['dwb'] = f32(np.stack([inputs['dw_b'][:128], inputs['dw_b'][128:]], 1))
    cons['lng'] = f32(np.stack([inputs['ln_g'][:128], inputs['ln_g'][128:]], 1))
    cons['lnb'] = f32(np.stack([inputs['ln_b'][:128], inputs['ln_b'][128:]], 1))
    s2 = inputs['bn2_g'] / np.sqrt(inputs['bn2_v'] + 1e-5)
    b2 = inputs['bn2_b'] - inputs['bn2_m'] * s2 + s2 * inputs['out_b']
    cons['bn2s'] = f32(np.stack([s2[:128], s2[128:]], 1))
    cons['bn2b'] = f32(np.stack([b2[:128], b2[128:]], 1))

    off_w = np.asarray(inputs['off_w'], np.float32)
    off_b = np.asarray(inputs['off_b'], np.float32)
    mk_w = np.asarray(inputs['mk_w'], np.float32)
    mk_b = np.asarray(inputs['mk_b'], np.float32)
    KS = np.arange(-3, 4, dtype=np.float32)

    wg1m63 = np.zeros((C, 63), np.float32)
    bg1m63 = np.zeros(63, np.float32)
    wg1t63 = np.zeros((C, 63), np.float32)
    bg1t63 = np.zeros(63, np.float32)
    for ky in range(NT):
        for pp in range(9):
            r = ky * 9 + pp
            wg1m63[:, r] = mk_w[:, pp]
            bg1m63[r] = mk_b[pp]
            wg1t63[:, r] = off_w[:, 2 * pp + 1]
            bg1t63[r] = off_b[2 * pp + 1] - KS[ky]
    wg1m = np.concatenate([wg1m63, wg1m63], 1)
    bg1m = np.concatenate([bg1m63, bg1m63])
    wg1t = np.concatenate([wg1t63, wg1t63], 1)
    bg1t = np.concatenate([bg1t63, bg1t63])
    cons['wg1m'] = np.zeros((128, 2 * 126), np.float32)
    cons['wg1t'] = np.zeros((128, 2 * 126), np.float32)
    for cic in range(2):
        cons['wg1m'][:, cic * 126:(cic + 1) * 126] = wg1m[cic * 128:(cic + 1) * 128]
        cons['wg1t'][:, cic * 126:(cic + 1) * 126] = wg1t[cic * 128:(cic + 1) * 128]
    cons['bg1m'] = f32(np.pad(bg1m, (0, 2))[:, None])
    cons['bg1t'] = f32(np.pad(bg1t, (0, 2))[:, None])

    wg2 = np.zeros((C, 441), np.float32)
    bg2 = np.zeros(441, np.float32)
    for kx in range(NT):
        for j in range(NT):
            for pp in range(9):
                r = kx * 63 + j * 9 + pp
                wg2[:, r] = off_w[:, 2 * pp]
                bg2[r] = off_b[2 * pp] - KS[kx]
    cons['wg2'] = np.zeros((128, 2 * 441), np.float32)
    for cic in range(2):
        cons['wg2'][:, cic * 441:(cic + 1) * 441] = wg2[cic * 128:(cic + 1) * 128]
    bg2p = np.zeros((128, 4), np.float32)
    for ch in range(4):
        c0, c1 = ch * 126, min(ch * 126 + 126, 441)
        bg2p[:c1 - c0, ch] = bg2[c0:c1]
    cons['bg2'] = bg2p

    S = np.zeros((441, 81), np.float32)
    for kx in range(NT):
        for ky in range(NT):
            for pp in range(9):
                dx, dy = pp // 3 - 1, pp % 3 - 1   # reference tap order
                u = dy + (ky - 3) + 4
                v = dx + (kx - 3) + 4
                S[kx * 63 + ky * 9 + pp, u * NB + v] = 1.0
    ssm = np.zeros((128, 4 * 81), np.float32)
    for ch in range(4):
        c0, c1 = ch * 126, min(ch * 126 + 126, 441)
        ssm[:c1 - c0, ch * 81:(ch + 1) * 81] = S[c0:c1]
    cons['ssm'] = ssm

    cons['onesA'] = np.full((128, 1), 1.0 / C, np.float32)
    cons['ones1'] = np.ones((128, 128), np.float32)
    return cons


def core_inputs(x, n, r0):
    xs = np.zeros((C, XR, WPAD), np.float32)
    lo, hi = r0 - 5, r0 + 21
    clo, chi = max(lo, 0), min(hi, H)
    xs[:, clo - lo:chi - lo, 4:68] = x[n, :, clo:chi, :]
    xsh = np.zeros((C, XBUF), np.float32)
    xsh[:, 1:1 + XF] = xs.reshape(C, XF)
    ym = np.zeros((YR, WPAD), np.float32)
    for b in range(YR):
        if 0 <= r0 - 4 + b < H:
            ym[b, 4:68] = 1.0
    return {'xsh': xsh, 'ymask': np.ascontiguousarray(ym.reshape(1, YF))}


IN_SHAPES = {
    'xsh': (256, XBUF), 'ymask': (1, YF),
    'w1t': (128, 4608), 'w2c': (128, 512), 'inbw2': (1, 256),
    'dww': (128, 18), 'dwb': (128, 2), 'bn1s': (128, 2), 'bn1b': (128, 2),
    'lng': (128, 2), 'lnb': (128, 2), 'bn2s': (128, 2), 'bn2b': (128, 2),
    'wg1m': (128, 252), 'wg1t': (128, 252), 'bg1m': (128, 1), 'bg1t': (128, 1),
    'wg2': (128, 882), 'bg2': (128, 4), 'ssm': (128, 324),
    'onesA': (128, 1), 'ones1': (128, 128),
}


# ---------------------------------------------------------------- kernel IR
@with_exitstack
def dcn_kernel(ctx: ExitStack, tc: tile.TileContext, outs, ins):
    nc = tc.nc
    CHUNK = 432            # cv1/xpw psum chunk (6 rows of 72)
    NCH = YF // CHUNK      # 4
    PXC = 512              # stage-C px chunk
    out_dram = outs['out']

    cpool = ctx.enter_context(tc.tile_pool(name="consts", bufs=1))
    wpool = ctx.enter_context(tc.tile_pool(name="work", bufs=1))
    spool = ctx.enter_context(tc.tile_pool(name="small", bufs=2))
    ps_main = ctx.enter_context(tc.tile_pool(name="psmain", bufs=5, space="PSUM"))
    ps_stat = ctx.enter_context(tc.tile_pool(name="psstat", bufs=2, space="PSUM"))

    def cload(name, shape):
        t = cpool.tile(shape, FP, name=name, tag=name)
        nc.sync.dma_start(t[:], ins[name][:, :])
        return t

    w1t = cload('w1t', [128, 4608])
    w2c = cload('w2c', [128, 512])
    inbw2 = cload('inbw2', [1, 256])
    dww = cload('dww', [128, 18])
    dwb = cload('dwb', [128, 2])
    bn1s = cload('bn1s', [128, 2])
    bn1b = cload('bn1b', [128, 2])
    lng = cload('lng', [128, 2])
    lnb = cload('lnb', [128, 2])
    bn2s = cload('bn2s', [128, 2])
    bn2b = cload('bn2b', [128, 2])
    wg1m = cload('wg1m', [128, 252])
    wg1t = cload('wg1t', [128, 252])
    bg1m = cload('bg1m', [128, 1])
    bg1t = cload('bg1t', [128, 1])
    wg2 = cload('wg2', [128, 882])
    bg2 = cload('bg2', [128, 4])
    ssm = cload('ssm', [128, 324])
    onesA = cload('onesA', [128, 1])
    ones1 = cload('ones1', [128, 128])
    ymask = cpool.tile([1, YF], FP, name='ymask', tag='ymask')
    nc.sync.dma_start(ymask[:], ins['ymask'][:, :])

    ident = cpool.tile([128, 128], FP, name='ident', tag='ident')
    make_identity(nc, ident[:])
    epsc = cpool.tile([128, 1], FP, name='epsc', tag='epsc')
    nc.gpsimd.memset(epsc[:], 1e-5)
    onec = cpool.tile([128, 1], FP, name='onec', tag='onec')
    nc.gpsimd.memset(onec[:], 1.0)
    zeroc = cpool.tile([128, 1], FP, name='zeroc', tag='zeroc')
    nc.gpsimd.memset(zeroc[:], 0.0)
    c447 = cpool.tile([128, 1], FP, name='c447', tag='c447')
    nc.gpsimd.memset(c447[:], 0.044715)
    halfc = cpool.tile([128, 1], FP, name='halfc', tag='halfc')
    nc.gpsimd.memset(halfc[:], 0.5)

    x2 = []
    for g in range(2):
        t = wpool.tile([128, XBUF], FP, name=f'x2_{g}', tag=f'x2_{g}')
        nc.sync.dma_start(t[:], ins['xsh'][g * 128:(g + 1) * 128, :])
        x2.append(t)

    xpw_pm = nc.dram_tensor('xpw_pm', [YF, 256], FP, kind='Internal')
    mdram = nc.dram_tensor('mdram', [NTILE * 128 * QW], FP, kind='Internal')

    # ================= stage A: cv1 + BN/SiLU + ymask =================
    y_sb = [wpool.tile([128, YF], FP, name=f'y_{g}', tag=f'y_{g}') for g in range(2)]
    for ch in range(NCH):
        co0 = ch * CHUNK
        mb = ps_main.tile([128, CHUNK], FP, name='mb', tag='mm')
        nc.tensor.matmul(mb[:], lhsT=ones1[0:1, :], rhs=ymask[0:1, co0:co0 + CHUNK],
                         start=True, stop=True)
        for g in range(2):
            acc = ps_main.tile([128, CHUNK], FP, name='acc', tag='mm')
            nmm = 18
            i = 0
            for tap in range(9):
                sh = (tap // 3) * WPAD + (tap % 3 - 1)
                for cic in range(2):
                    nc.tensor.matmul(
                        acc[:],
                        lhsT=w1t[:, (tap * 2 + cic) * 256 + g * 128:
                                 (tap * 2 + cic) * 256 + g * 128 + 128],
                        rhs=x2[cic][:, 1 + sh + co0: 1 + sh + co0 + CHUNK],
                        start=(i == 0), stop=(i == nmm - 1))
                    i += 1
            tmp = spool.tile([128, CHUNK], FP, name='ytmp', tag='ytmp')
            nc.vector.tensor_scalar(out=tmp[:], in0=acc[:],
                                    scalar1=bn1s[:, g:g + 1],
                                    scalar2=bn1b[:, g:g + 1],
                                    op0=OP.mult, op1=OP.add)
            sg = spool.tile([128, CHUNK], FP, name='sg', tag='ytmp2', bufs=2)
            nc.scalar.activation(sg[:], tmp[:], AF.Sigmoid, bias=zeroc[:, :])
            sv = spool.tile([128, CHUNK], FP, name='sv', tag='ytmp2', bufs=2)
            nc.vector.tensor_tensor(sv[:], tmp[:], sg[:], op=OP.mult)
            nc.vector.tensor_tensor(y_sb[g][:, co0:co0 + CHUNK], sv[:], mb[:],
                                    op=OP.mult)

    # ================= stage B: xpw = y@W2 + inbW2 (x) ymask ==========
    xpw_cm = [wpool.tile([128, YF], FP, name=f'xpw_{g}', tag=f'xpw_{g}')
              for g in range(2)]
    for ch in range(NCH):
        co0 = ch * CHUNK
        for g in range(2):
            acc = ps_main.tile([128, CHUNK], FP, name='acc2', tag='mm')
            for cic in range(2):
                nc.tensor.matmul(acc[:],
                                 lhsT=w2c[:, cic * 256 + g * 128: cic * 256 + g * 128 + 128],
                                 rhs=y_sb[cic][:, co0:co0 + CHUNK],
                                 start=(cic == 0), stop=False)
            nc.tensor.matmul(acc[:], lhsT=inbw2[0:1, g * 128:g * 128 + 128],
                             rhs=ymask[0:1, co0:co0 + CHUNK], start=False, stop=True)
            nc.scalar.copy(xpw_cm[g][:, co0:co0 + CHUNK], acc[:])

    # ---- stage B2: transpose xpw to pixel-major, write to DRAM ----
    stg = [wpool.tile([128, 14 * 128], FP, name=f'stg_{g}', tag=f'y_{g}')
           for g in range(2)]
    for g in range(2):
        for b in range(14):
            p0 = b * 128
            w = min(128, YF - p0)
            tp = ps_main.tile([128, 128], FP, name='tp', tag='mm')
            nc.tensor.transpose(tp[0:w, :], in_=xpw_cm[g][:, p0:p0 + w],
                                identity=ident[:])
            nc.scalar.copy(stg[g][0:w, b * 128:b * 128 + 128], tp[0:w, :])
    for g in range(2):
        s3 = stg[g][:].rearrange("p (b c) -> p b c", c=128)
        dst = bass.AP(tensor=xpw_pm, offset=g * 128,
                      ap=[[256, 128], [128 * 256, 13], [1, 128]])
        nc.sync.dma_start(out=dst, in_=s3[:, 0:13, :])
        dst2 = bass.AP(tensor=xpw_pm, offset=13 * 128 * 256 + g * 128,
                       ap=[[256, 64], [1, 128]])
        nc.sync.dma_start(out=dst2, in_=stg[g][0:64, 13 * 128:14 * 128])

    # ================= stage C: dw conv + LN + GELU ====================
    x1 = [wpool.tile([128, PX], FP, name=f'x1_{g}', tag=f'x1_{g}') for g in range(2)]
    for g in range(2):
        yr = y_sb[g][:].rearrange("p (r w) -> p r w", w=WPAD)
        xr = x1[g][:].rearrange("p (r w) -> p r w", w=64)
        first = True
        for tap in range(9):
            ky, kx = tap // 3, tap % 3
            src = yr[:, 3 + ky:3 + ky + ROWS, 3 + kx:3 + kx + 64]
            if first:
                nc.scalar.activation(xr[:, :, :], src, AF.Identity,
                                     bias=dwb[:, g:g + 1],
                                     scale=dww[:, g * 9 + tap:g * 9 + tap + 1])
                first = False
            else:
                nc.vector.scalar_tensor_tensor(
                    out=xr[:, :, :], in0=src,
                    scalar=dww[:, g * 9 + tap:g * 9 + tap + 1],
                    in1=xr[:, :, :], op0=OP.mult, op1=OP.add)

    sq = [wpool.tile([128, PX], FP, name=f'sq_{g}', tag=f'sq_{g}') for g in range(2)]
    for g in range(2):
        nc.scalar.activation(sq[g][:], x1[g][:], AF.Square, bias=zeroc[:, :])

    x1n = [wpool.tile([128, PX], FP, name=f'x1n_{g}', tag=f'x1n_{g}')
           for g in range(2)]
    for pc in range(PX // PXC):
        p0 = pc * PXC
        mu = ps_stat.tile([1, PXC], FP, name='mu', tag='stat')
        for g in range(2):
            nc.tensor.matmul(mu[:], lhsT=onesA[:, :], rhs=x1[g][:, p0:p0 + PXC],
                             start=(g == 0), stop=(g == 1))
        sqm = ps_stat.tile([1, PXC], FP, name='sqm', tag='stat')
        for g in range(2):
            nc.tensor.matmul(sqm[:], lhsT=onesA[:, :], rhs=sq[g][:, p0:p0 + PXC],
                             start=(g == 0), stop=(g == 1))
        mu_sb = spool.tile([1, PXC], FP, name='mu_sb', tag='mu_sb')
        nc.scalar.copy(mu_sb[:], mu[:])
        mu2 = spool.tile([1, PXC], FP, name='mu2', tag='mu2')
        nc.vector.tensor_tensor(mu2[:], mu_sb[:], mu_sb[:], op=OP.mult)
        var = spool.tile([1, PXC], FP, name='var', tag='var')
        nc.vector.tensor_tensor(var[:], sqm[:], mu2[:], op=OP.subtract)
        sd = spool.tile([1, PXC], FP, name='sd', tag='sd')
        nc.scalar.activation(sd[:], var[:], AF.Sqrt, bias=epsc[0:1, :], scale=1.0)
        rstd = spool.tile([1, PXC], FP, name='rstd', tag='rstd')
        nc.vector.reciprocal(rstd[:], sd[:])
        mub = ps_main.tile([128, PXC], FP, name='mub', tag='mm')
        nc.tensor.matmul(mub[:], lhsT=ones1[0:1, :], rhs=mu_sb[:, :],
                         start=True, stop=True)
        rsb = ps_main.tile([128, PXC], FP, name='rsb', tag='mm')
        nc.tensor.matmul(rsb[:], lhsT=ones1[0:1, :], rhs=rstd[:, :],
                         start=True, stop=True)
        for g in range(2):
            t1 = spool.tile([128, PXC], FP, name='t1', tag='gtmp', bufs=3)
            nc.vector.tensor_tensor(t1[:], x1[g][:, p0:p0 + PXC], mub[:],
                                    op=OP.subtract)
            t2 = spool.tile([128, PXC], FP, name='t2', tag='gtmp', bufs=3)
            nc.vector.tensor_tensor(t2[:], t1[:], rsb[:], op=OP.mult)
            tg = spool.tile([128, PXC], FP, name='tg', tag='tg')
            nc.vector.tensor_scalar(out=tg[:], in0=t2[:],
                                    scalar1=lng[:, g:g + 1],
                                    scalar2=lnb[:, g:g + 1],
                                    op0=OP.mult, op1=OP.add)
            u2 = spool.tile([128, PXC], FP, name='u2', tag='gtmp', bufs=3)
            nc.scalar.activation(u2[:], tg[:], AF.Square, bias=zeroc[:, :])
            s3 = spool.tile([128, PXC], FP, name='s3', tag='gtmp', bufs=3)
            nc.vector.tensor_scalar(out=s3[:], in0=u2[:], scalar1=c447[:, :],
                                    scalar2=onec[:, :], op0=OP.mult, op1=OP.add)
            a3 = spool.tile([128, PXC], FP, name='a3', tag='gtmp', bufs=3)
            nc.vector.tensor_tensor(a3[:], s3[:], tg[:], op=OP.mult)
            th = spool.tile([128, PXC], FP, name='th', tag='gtmp', bufs=3)
            nc.scalar.activation(th[:], a3[:], AF.Tanh, bias=zeroc[:, :],
                                 scale=0.7978845608028654)
            q3 = spool.tile([128, PXC], FP, name='q3', tag='gtmp', bufs=3)
            nc.vector.tensor_scalar(out=q3[:], in0=th[:], scalar1=halfc[:, :],
                                    scalar2=halfc[:, :], op0=OP.mult, op1=OP.add)
            nc.vector.tensor_tensor(x1n[g][:, p0:p0 + PXC], q3[:], tg[:],
                                    op=OP.mult)

    # ================= stage C2: G1/G2, tents, K bins ==================
    kn_sb = wpool.tile([81, PX], FP, name='kn', tag='kn')
    for pc in range(PX // PXC):
        p0 = pc * PXC
        g1m = ps_main.tile([126, PXC], FP, name='g1m', tag='mm')
        for cic in range(2):
            nc.tensor.matmul(g1m[:], lhsT=wg1m[:, cic * 126:(cic + 1) * 126],
                             rhs=x1n[cic][:, p0:p0 + PXC],
                             start=(cic == 0), stop=(cic == 1))
        g1t = ps_main.tile([126, PXC], FP, name='g1t', tag='mm')
        for cic in range(2):
            nc.tensor.matmul(g1t[:], lhsT=wg1t[:, cic * 126:(cic + 1) * 126],
                             rhs=x1n[cic][:, p0:p0 + PXC],
                             start=(cic == 0), stop=(cic == 1))
        m_sb = spool.tile([126, PXC], FP, name='m_sb', tag='m_sb')
        nc.scalar.activation(m_sb[:], g1m[:], AF.Exp, bias=bg1m[0:126, :], scale=1.0)
        tyab = spool.tile([126, PXC], FP, name='tyab', tag='ttmp', bufs=4)
        nc.scalar.activation(tyab[:], g1t[:], AF.Abs, bias=bg1t[0:126, :], scale=1.0)
        ty = spool.tile([126, PXC], FP, name='ty', tag='ttmp', bufs=4)
        nc.scalar.activation(ty[:], tyab[:], AF.Relu, bias=onec[0:126, :], scale=-1.0)
        A = spool.tile([126, PXC], FP, name='A', tag='A')
        nc.vector.tensor_tensor(A[:], m_sb[:], ty[:], op=OP.mult)

        kps = ps_main.tile([81, PXC], FP, name='kps', tag='mm')
        for chn in range(4):
            r0c, r1c = chn * 126, min(chn * 126 + 126, 441)
            rows = r1c - r0c
            g2 = ps_main.tile([126, PXC], FP, name='g2', tag='mm')
            for cic in range(2):
                nc.tensor.matmul(g2[0:rows, :],
                                 lhsT=wg2[:, cic * 441 + r0c: cic * 441 + r1c],
                                 rhs=x1n[cic][:, p0:p0 + PXC],
                                 start=(cic == 0), stop=(cic == 1))
            txab = spool.tile([126, PXC], FP, name='txab', tag='ttmp', bufs=4)
            nc.scalar.activation(txab[0:rows, :], g2[0:rows, :], AF.Abs,
                                 bias=bg2[0:rows, chn:chn + 1], scale=1.0)
            tx = spool.tile([126, PXC], FP, name='tx', tag='ttmp', bufs=4)
            nc.scalar.activation(tx[0:rows, :], txab[0:rows, :], AF.Relu,
                                 bias=onec[0:rows, :], scale=-1.0)
            P = spool.tile([126, PXC], FP, name='P', tag='ttmp', bufs=4)
            nc.vector.tensor_tensor(P[0:rows, :], A[0:rows, :], tx[0:rows, :],
                                    op=OP.mult)
            nc.tensor.matmul(kps[:], lhsT=ssm[0:rows, chn * 81:(chn + 1) * 81],
                             rhs=P[0:rows, :], start=(chn == 0), stop=(chn == 3))
        den = ps_stat.tile([1, PXC], FP, name='den', tag='stat')
        nc.tensor.matmul(den[:], lhsT=ones1[0:9, 0:1], rhs=m_sb[0:9, :],
                         start=True, stop=True)
        dsb = spool.tile([1, PXC], FP, name='dsb', tag='dsb')
        nc.scalar.copy(dsb[:], den[:])
        rec = spool.tile([1, PXC], FP, name='rec', tag='rec')
        nc.vector.reciprocal(rec[:], dsb[:])
        sbp = ps_main.tile([81, PXC], FP, name='sbp', tag='mm')
        nc.tensor.matmul(sbp[:], lhsT=ones1[0:1, 0:81], rhs=rec[:, :],
                         start=True, stop=True)
        kraw = spool.tile([81, PXC], FP, name='kraw', tag='kraw', bufs=1)
        nc.scalar.copy(kraw[:], kps[:])
        nc.vector.tensor_tensor(kn_sb[:, p0:p0 + PXC], kraw[:], sbp[:], op=OP.mult)

    kt_sb = wpool.tile([128, NTILE * 81], FP, name='kt', tag='kt')
    for t in range(NTILE):
        tp = ps_main.tile([128, 128], FP, name='tpk', tag='mm')
        nc.tensor.transpose(tp[:, 0:81], in_=kn_sb[0:81, t * 128:(t + 1) * 128],
                            identity=ident[0:81, 0:81])
        nc.scalar.copy(kt_sb[:, t * 81:(t + 1) * 81], tp[:, 0:81])

    # ================= stage D: M build + blend =======================
    zero720 = cpool.tile([128, QW], FP, name='zero720', tag='zero720')
    nc.gpsimd.memset(zero720[:], 0.0)
    for t in range(NTILE):
        dst = bass.AP(tensor=mdram, offset=t * 128 * QW, ap=[[QW, 128], [1, QW]])
        nc.scalar.dma_start(out=dst, in_=zero720[:])
    for u in range(NB):
        for half in range(2):
            src = kt_sb[half * 64:half * 64 + 64, :].rearrange(
                "p (t uv) -> p t uv", uv=81)[:, :, u * 9:u * 9 + 9]
            off = (half * 64) * QW + (half + u) * WPAD
            dst = bass.AP(tensor=mdram, offset=off,
                          ap=[[QW + 1, 64], [128 * QW, NTILE], [1, 9]])
            nc.sync.dma_start(out=dst, in_=src)

    out_sb = [wpool.tile([128, PX], FP, name=f'out_{g}', tag=f'xpw_{g}')
              for g in range(2)]
    for t in range(NTILE):
        m_sb_t = wpool.tile([128, QW], FP, name='m_t', tag='x1_0')
        msrc = bass.AP(tensor=mdram, offset=t * 128 * QW, ap=[[QW, 128], [1, QW]])
        nc.scalar.dma_start(out=m_sb_t[:], in_=msrc)
        mt = wpool.tile([128, 6 * 128], FP, name='mt', tag='x1_1')
        for qc in range(6):
            q0 = qc * 128
            w = min(128, QW - q0)
            tp = ps_main.tile([128, 128], FP, name='tpm', tag='mm')
            nc.tensor.transpose(tp[0:w, :], in_=m_sb_t[:, q0:q0 + w],
                                identity=ident[:])
            nc.scalar.copy(mt[0:w, qc * 128:qc * 128 + 128], tp[0:w, :])
        win = spool.tile([128, 6 * 256], FP, name='win', tag='win', bufs=1)
        winr = win[:].rearrange("p (b c) -> p b c", c=256)
        wsrc = bass.AP(tensor=xpw_pm, offset=t * 144 * 256,
                       ap=[[256, 128], [128 * 256, 5], [1, 256]])
        nc.sync.dma_start(out=winr[:, 0:5, :], in_=wsrc)
        wsrc2 = bass.AP(tensor=xpw_pm, offset=(t * 144 + 640) * 256,
                        ap=[[256, 80], [1, 256]])
        nc.sync.dma_start(out=winr[0:80, 5, :], in_=wsrc2)
        for g in range(2):
            zps = ps_main.tile([128, 128], FP, name='zps', tag='mm')
            for qc in range(6):
                w = min(128, QW - qc * 128)
                nc.tensor.matmul(zps[:],
                                 lhsT=winr[0:w, qc, g * 128:g * 128 + 128],
                                 rhs=mt[0:w, qc * 128:qc * 128 + 128],
                                 start=(qc == 0), stop=(qc == 5))
            zt = spool.tile([128, 128], FP, name='zt', tag='zt')
            nc.vector.tensor_scalar(out=zt[:], in0=zps[:],
                                    scalar1=bn2s[:, g:g + 1],
                                    scalar2=bn2b[:, g:g + 1],
                                    op0=OP.mult, op1=OP.add)
            zg = spool.tile([128, 128], FP, name='zg', tag='zg')
            nc.scalar.activation(zg[:], zt[:], AF.Sigmoid, bias=zeroc[:, :])
            zs = spool.tile([128, 128], FP, name='zs', tag='zs')
            nc.vector.tensor_tensor(zs[:], zt[:], zg[:], op=OP.mult)
            res = x2[g][:, 1:1 + XF].rearrange("p (r w) -> p r w", w=WPAD)[
                :, 5 + 2 * t:7 + 2 * t, 4:68]
            zsr = zs[:].rearrange("p (a b) -> p a b", b=64)
            outr = out_sb[g][:, t * 128:(t + 1) * 128].rearrange(
                "p (a b) -> p a b", b=64)
            nc.vector.tensor_tensor(outr, zsr, res, op=OP.add)

    for g in range(2):
        nc.sync.dma_start(out=out_dram[g * 128:(g + 1) * 128, :], in_=out_sb[g][:])


# ---------------------------------------------------------------- driver
_CACHED_NC = None


def _build_nc():
    global _CACHED_NC
    if _CACHED_NC is not None:
        return _CACHED_NC
    nc = bacc.Bacc("TRN2", target_bir_lowering=False, debug=False, num_devices=8)
    ins = {}
    for name, shape in IN_SHAPES.items():
        ins[name] = nc.dram_tensor(name, list(shape), FP, kind='ExternalInput').ap()
    out_ap = nc.dram_tensor('out', [256, PX], FP, kind='ExternalOutput').ap()
    with tile.TileContext(nc) as tc:
        dcn_kernel(tc, {'out': out_ap}, ins)
    nc.compile()
    _CACHED_NC = nc
    return nc


def kernel(**inputs):
    global LAST_EXEC_NS
    inputs = {k: np.asarray(v) for k, v in inputs.items()}
    x = np.asarray(inputs['x'], np.float32)
    cons = host_consts(inputs)
    in_maps = []
    shards = []
    for core in range(8):
        n, r0 = core // 4, (core % 4) * 16
        shards.append((n, r0))
        im = dict(cons)
        im.update(core_inputs(x, n, r0))
        in_maps.append(im)

    nc = _build_nc()
    res = run_bass_kernel_spmd(nc, in_maps, core_ids=list(range(8)))
    global LAST_RESULTS
    LAST_RESULTS = res
    LAST_EXEC_NS = res.exec_time_ns

    out = np.zeros((N, C, H, W), np.float32)
    for core, (n, r0) in enumerate(shards):
        out[n, :, r0:r0 + 16, :] = res.results[core]['out'].reshape(C, ROWS, 64)
    return out
